# revision 36
# baseline (speedup 1.0000x reference)
# Trainium2 Bass kernel for nn_Actor (gnn_message_passing), 8-core data parallel.
#
# Math (per batch row b):
#   att = sigmoid(g @ W_cast + b_cast)                      [50]
#   x_n = concat(body(20), obj_n(30)) * att                 [50] per object n<8
#   h_n = relu(x_n @ W_a0 + b_a0)                           [256]
#   y_n = relu(h_n @ W_a1 + b_a1)                           [400]
#   pi  = sum_n y_n                                         [400]
#   out = tanh(relu(relu(pi@W_p0+b_p0)@W_p1+b_p1)@W_p2+b_p2)  [8]
#
# Mapping: feature-major on-chip ([feature partitions, batch free]).
# a1 splits by output column: 0:128 runs bf16-W x fp8-h, 128:384 runs two
# fp8 DoubleRow MMs (K=256 in one pass), 384:400 (tails) run as col-tiled
# M=16 fp8 MMs packed 4-objects-per-PSUM-bank so their relu costs 1/4.
# fp8 a1 weights use 8 balanced-rounding copies (one per object) so the
# quantization error cancels in the deepset sum; the p0 tail weights are a
# balanced fp8 pair across the two ttail banks. pi accumulates via a
# custom fused DVE op out = relu(psum*s) + pi (object 0 seeds from a
# zeros tile so no ACT work sits on the matmul-gating path). Per object
# the DR MMs run first so the single-buffered yA bank has DVE drain
# slack. The packed tail banks and the pi chunks enter p0 directly
# (tails as one fp8-DR K-chunk pair); p0/p1 biases ride the ACT relu
# bias port. The p-chain for tile t runs during tile t+1; att/gating for
# t+2 is emitted mid-tile so gpsimd gating is never just-in-time. xsrc
# is partition-major [128,4,bsh] so each tile's inputs are one DMA, and
# DMA issue order follows first use to shorten the prologue.
import numpy as np
import ml_dtypes

BF16 = ml_dtypes.bfloat16
FP8 = ml_dtypes.float8_e4m3   # TRN FP8_EXP4 (inf at 256, max 240)

WA1_S = 16.0                  # a1 weight pre-scale into fp8/bf16 grid
H_S = 4.0                     # h pre-scale into fp8 grid
Y_DESCALE = 1.0 / (WA1_S * H_S)
TT_S = 4.0                    # packed-tail (relu'd y) fp8 pre-scale
WP0_S = 16.0                  # p0 weight pre-scale

B = 65536
NCORES = 8
BSH = B // NCORES          # 8192 rows per core
TN = 512                   # batch tile (matmul free dim / psum bank)
BODY = 10
FEAT = 15
NOBJ = 8
HALF = 130

_BODY_COLS = list(range(0, 10)) + list(range(130, 140))


def _obj_cols(n):
    return list(range(10 + 15 * n, 25 + 15 * n)) + list(range(140 + 15 * n, 155 + 15 * n))


# ---------------------------------------------------------------- host packing

def _fp8_rtn(x):
    return np.clip(x, -240.0, 240.0).astype(FP8).astype(np.float32)


def _balanced_fp8_copies(f, ncopies):
    """ncopies fp8 quantizations of f whose per-element sum ~= ncopies*f.

    Per element: split copies between the two fp8 neighbours of f in
    proportion to the residual, assigning which copy rounds away via a random
    permutation (fixed seed) so the per-object errors decorrelate from the
    object index."""
    rng = np.random.default_rng(23)
    q = _fp8_rtn(f)
    r = (f - q).astype(np.float32)
    aq = np.abs(q)
    e = np.floor(np.log2(np.maximum(aq, 2.0 ** -6)))
    ulp = (2.0 ** (e - 3)).astype(np.float32)
    alt = _fp8_rtn(q + np.sign(r) * ulp * 1.001)
    denom = np.abs(alt - q)
    frac = np.where(denom > 0, np.abs(r) / np.maximum(denom, 1e-30), 0.0)
    frac = np.clip(frac, 0.0, 1.0)
    k = np.rint(frac * ncopies)
    order = rng.random((ncopies,) + f.shape).argsort(axis=0)
    take_alt = order < k[None]
    out = np.where(take_alt, alt[None], q[None])
    return out.astype(FP8)


def _pack_weights(W_cast, b_cast, W_a0, b_a0, W_a1, b_a1,
                  W_p0, b_p0, W_p1, b_p1, W_p2, b_p2):
    f32 = np.float32
    # attention weights, column-replicated into the gate layout:
    # psum rows 0:50 = att[0:50] (object A gate), rows 64:114 = same (object B),
    # row 50/114 -> constant 1.0 after sigmoid (bias 30).
    wcast = np.zeros((100, 128), f32)
    wcast[:, 0:50] = W_cast
    wcast[:, 64:114] = W_cast
    bcast = np.zeros((128, 1), f32)
    bcast[0:50, 0] = b_cast
    bcast[64:114, 0] = b_cast
    bcast[50, 0] = 30.0
    bcast[114, 0] = 30.0

    # a0: K=51 (50 features + const-1 row carrying b_a0), M=256 in two chunks.
    wa0 = np.zeros((128, 2, 128), f32)
    for c in range(2):
        wa0[0:50, c, :] = W_a0[:, 128 * c:128 * (c + 1)]
        wa0[50, c, :] = b_a0[128 * c:128 * (c + 1)]
        wa0[64:114, c, :] = W_a0[:, 128 * c:128 * (c + 1)]
        wa0[114, c, :] = b_a0[128 * c:128 * (c + 1)]

    # a1, all pre-scaled by WA1_S (h carries H_S; psum = 64*y):
    #   wa1   bf16 [128, 2(kc), 256]   output cols 0:256
    #   wa1dr fp8  [8, 128, 2, 128]    cols 256:384, per-object balanced
    #   wtail fp8  [8, 128, 2, 16]     cols 384:400, per-object balanced
    wa1 = np.stack([W_a1[0:128, 0:128], W_a1[128:256, 0:128]], 1) * WA1_S
    drf = np.stack([W_a1[0:128, 128:384], W_a1[128:256, 128:384]], 1) * WA1_S
    tlf = np.zeros((128, 2, 32), f32)
    tlf[:, 0, 0:16] = W_a1[0:128, 384:400] * WA1_S
    tlf[:, 1, 0:16] = W_a1[128:256, 384:400] * WA1_S
    wa1dr = _balanced_fp8_copies(drf, NOBJ)
    wtail = _balanced_fp8_copies(tlf, NOBJ)

    # a1 biases for the (rare) nonzero-bias fallback:
    # ba1 [128, 3]: per-bank bias columns for y cols 0:128,128:256,256:384.
    ba1 = np.zeros((128, 3), f32)
    ba1[:, 0] = b_a1[0:128]
    ba1[:, 1] = b_a1[128:256]
    ba1[:, 2] = b_a1[256:384]
    # tail bias, replicated into the 4 col strips, pre-scaled by TT_S
    ba1t = np.zeros((128, 1), f32)
    for k in range(4):
        ba1t[32 * k:32 * k + 16, 0] = b_a1[384:400] * TT_S

    # p0: pi chunks (K=384) in bf16 (x WP0_S); tail rows via fp8 DoubleRow
    # on the packed tail banks (x WP0_S / TT_S); bias via ACT.
    wp0 = np.zeros((128, 3, 256), f32)
    wp0[:, 0, :] = W_p0[0:128, :]
    wp0[:, 1, :] = W_p0[128:256, :]
    wp0[:, 2, :] = W_p0[256:384, :]
    wp0 = wp0 * WP0_S
    # wp0t banks 0/1 multiply tt0/tt1 whose distributions match, so a
    # balanced fp8 pair cancels most of the tail-weight quantization error.
    tpair = _balanced_fp8_copies(W_p0[384:400, :] * (WP0_S / TT_S), 2)
    tpair = tpair.astype(f32)
    wp0t = np.zeros((128, 2, 256), f32)
    for k in range(4):
        wp0t[32 * k:32 * k + 16, 0, :] = tpair[0]
        wp0t[32 * k:32 * k + 16, 1, :] = tpair[1]
    bp0 = np.zeros((128, 2), f32)
    bp0[:, 0] = b_p0[0:128]
    bp0[:, 1] = b_p0[128:256]

    wp1 = np.zeros((128, 2, 256), f32)
    wp1[:, 0, :] = W_p1[0:128, :]
    wp1[:, 1, :] = W_p1[128:256, :]
    bp1 = np.zeros((128, 2), f32)
    bp1[:, 0] = b_p1[0:128]
    bp1[:, 1] = b_p1[128:256]

    wp2 = np.zeros((128, 2, 8), f32)
    wp2[:, 0, :] = W_p2[0:128, :]
    wp2[:, 1, :] = W_p2[128:256, :]
    bp2 = np.asarray(b_p2, f32).reshape(8, 1)

    # single f32 bias blob [128, 10]: col 0 bcast, 1:4 ba1, 4 ba1t,
    # 5:7 bp0, 7:9 bp1, 9 bp2 (rows 0:8) -> one DMA instead of six
    bvec = np.zeros((128, 10), f32)
    bvec[:, 0:1] = bcast
    bvec[:, 1:4] = ba1
    bvec[:, 4:5] = ba1t
    bvec[:, 5:7] = bp0
    bvec[:, 7:9] = bp1
    bvec[0:8, 9:10] = bp2

    return {
        "wcast": wcast.astype(BF16), "bcast": bcast,
        "wa0": wa0.astype(BF16),
        "wa1": wa1.astype(BF16), "wa1dr": wa1dr, "wtail": wtail,
        # partition-major copies so each loads with a single DMA
        "wa1dr_dev": np.ascontiguousarray(wa1dr.transpose(1, 0, 2, 3)),
        "wtail_dev": np.ascontiguousarray(wtail.transpose(1, 0, 2, 3)),
        "bvec": bvec,
        "ba1": ba1, "ba1t": ba1t,
        "wp0": wp0.astype(BF16), "wp0t": _fp8_rtn(wp0t).astype(FP8),
        "bp0": bp0,
        "wp1": wp1.astype(BF16), "bp1": bp1,
        "wp2": wp2.astype(BF16), "bp2": bp2,
    }


def _pack_shard(o_sh, g_sh):
    """o_sh [bsh, 260] f32, g_sh [bsh, 100] f32 -> xsrc [128,4,bsh], gT [100,bsh] bf16.

    xsrc is partition-major so each batch tile loads with a single DMA."""
    bsh = o_sh.shape[0]
    oT = np.ascontiguousarray(o_sh.T)  # [260, bsh]
    xsrc = np.zeros((4, 128, bsh), np.float32)
    body = oT[_BODY_COLS]  # [20, bsh]
    for p in range(4):
        xsrc[p, 0:20] = body
        xsrc[p, 20:50] = oT[_obj_cols(2 * p)]
        xsrc[p, 50] = 1.0
        xsrc[p, 64:84] = body
        xsrc[p, 84:114] = oT[_obj_cols(2 * p + 1)]
        xsrc[p, 114] = 1.0
    xsrc = np.ascontiguousarray(xsrc.transpose(1, 0, 2))  # [128, 4, bsh]
    gT = np.ascontiguousarray(g_sh.T)  # [100, bsh]
    return {"xsrc": xsrc.astype(BF16), "gT": gT.astype(BF16)}


# ---------------------------------------------------------------- custom DVE op

_RELU_ADD_OP = None


def _register_relu_add():
    """Register a custom DVE op out = relu(in0*s0) + in1 (fused PSUM relu +
    deepset accumulate). Monkey-registers into concourse.dve_ops so the
    per-NEFF table generator finds it; uops_sha computed in-process."""
    global _RELU_ADD_OP
    if _RELU_ADD_OP is not None:
        return _RELU_ADD_OP
    import concourse.dve_ops as dve_ops
    from concourse.dve_spec import Spec, Src0, Src1, C0, relu, lower
    from concourse.dve_spec import _has_src1
    from concourse.dve_uop import DveOpSpec

    name = "RELU_SCALE_ADD_ANT"
    for op in dve_ops.OPS:
        if op.name == name:
            _RELU_ADD_OP = op
            return op
    spec = Spec(
        body=relu(Src0 * C0) + Src1,
        reference=lambda in0, in1, s0, s1, imm2: (
            np.maximum(np.asarray(in0, np.float32) * s0, 0.0)
            + np.asarray(in1, np.float32)),
    )
    row = dve_ops._CUSTOM_DVE_ROW_BASE + len(dve_ops.OPS)
    assert row < 0x20
    dve_ops._SUB_OPCODE_FOR_NAME[name] = row
    op = dve_ops.DveOp(name, spec, subdim=False, uops_sha={})
    for ver in ("v3", "v4"):
        s = DveOpSpec(name=name, opcode=row, uops=lower(spec, ver=ver),
                      rd1_en=_has_src1(spec))
        op.uops_sha[ver] = s.sha(ver)
    dve_ops.OPS.append(op)
    dve_ops.CUSTOM_DVE_SPECS[name] = spec
    _RELU_ADD_OP = op
    return op


# ---------------------------------------------------------------- bass kernel

def build_nc(bsh=BSH, zero_ba1=True, zero_bp=True):
    import concourse.bass as bass
    import concourse.mybir as mybir
    import concourse.tile as tile
    from concourse import bacc

    ra_op = _register_relu_add()

    f32 = mybir.dt.float32
    bf16 = mybir.dt.bfloat16
    f8 = mybir.dt.float8e4
    AF = mybir.ActivationFunctionType
    ALU = mybir.AluOpType
    DR = mybir.MatmulPerfMode.DoubleRow
    ds = bass.ds

    nt = bsh // TN
    nc = bacc.Bacc("TRN2", target_bir_lowering=False, debug=False)

    xsrc_d = nc.dram_tensor("xsrc", [128, 4, bsh], bf16, kind="ExternalInput")
    gT_d = nc.dram_tensor("gT", [100, bsh], bf16, kind="ExternalInput")
    wcast_d = nc.dram_tensor("wcast", [100, 128], bf16, kind="ExternalInput")
    bvec_d = nc.dram_tensor("bvec", [128, 10], f32, kind="ExternalInput")
    wa0_d = nc.dram_tensor("wa0", [128, 2, 128], bf16, kind="ExternalInput")
    wa1_d = nc.dram_tensor("wa1", [128, 2, 128], bf16, kind="ExternalInput")
    wa1dr_d = nc.dram_tensor("wa1dr", [NOBJ, 128, 2, 256], f8,
                             kind="ExternalInput")
    wtail_d = nc.dram_tensor("wtail", [NOBJ, 128, 2, 32], f8,
                             kind="ExternalInput")
    wp0_d = nc.dram_tensor("wp0", [128, 3, 256], bf16, kind="ExternalInput")
    wp0t_d = nc.dram_tensor("wp0t", [128, 2, 256], f8, kind="ExternalInput")
    wp1_d = nc.dram_tensor("wp1", [128, 2, 256], bf16, kind="ExternalInput")
    wp2_d = nc.dram_tensor("wp2", [128, 2, 8], bf16, kind="ExternalInput")
    out_d = nc.dram_tensor("out", [8, bsh], f32, kind="ExternalOutput")

    with tile.TileContext(nc) as tc:
        with (
            tc.tile_pool(name="s_w", bufs=1) as s_w,
            tc.tile_pool(name="s_in", bufs=3) as s_in,
            tc.tile_pool(name="s_x", bufs=3) as s_x,
            tc.tile_pool(name="s_h", bufs=4) as s_h,
            tc.tile_pool(name="s_pi", bufs=2) as s_pi,
            tc.tile_pool(name="s_tt", bufs=2) as s_tt,
            tc.tile_pool(name="s_p", bufs=2) as s_p,
            tc.tile_pool(name="s_o", bufs=2) as s_o,
            tc.tile_pool(name="p_yA", bufs=1, space="PSUM") as p_yA,
            tc.tile_pool(name="p_yAB", bufs=2, space="PSUM") as p_yAB,
            tc.tile_pool(name="p_h", bufs=1, space="PSUM") as p_h,
            tc.tile_pool(name="p_s", bufs=1, space="PSUM") as p_s,
        ):
            # --- tile-0/1 input DMAs go FIRST on the sync ring; weights ride
            # the scalar ring so the first att/gating work isn't queued
            # behind ~25 weight descriptors.
            def emit_in_dma(t):
                cs = ds(t * TN, TN)
                gt = s_in.tile([100, TN], bf16, tag="gt")
                nc.sync.dma_start(gt[:], gT_d[:, cs])
                xt = s_in.tile([128, 4, TN], bf16, tag="xt")
                nc.sync.dma_start(xt[:], xsrc_d[:, :, cs])
                return xt, gt

            def wtile(name, shape, dt, src):
                t = s_w.tile(shape, dt, tag=name)
                nc.sync.dma_start(t[:], src)
                return t

            # DMA issue order follows first use so the sync ring feeds the
            # prologue: att(t0) weights, t0 inputs, a0/a1 weights, t1 inputs,
            # remaining object weights, then the (tile-2+) p-chain weights.
            wcast = wtile("wcast", [100, 128], bf16, wcast_d[:, :])
            bvec = wtile("bvec", [128, 10], f32, bvec_d[:, :])
            pre_in = {0: emit_in_dma(0)}
            wa0 = wtile("wa0", [128, 2, 128], bf16, wa0_d[:, :, :])
            wa1 = wtile("wa1", [128, 2, 128], bf16, wa1_d[:, :, :])
            wa1dr = s_w.tile([128, NOBJ, 2, 256], f8, tag="wa1dr")
            wtail = s_w.tile([128, NOBJ, 2, 32], f8, tag="wtail")
            for n in range(2):
                nc.sync.dma_start(wa1dr[:, n, :, :], wa1dr_d[n, :, :, :])
                nc.sync.dma_start(wtail[:, n, :, :], wtail_d[n, :, :, :])
            pre_in[1] = emit_in_dma(1)
            for n in range(2, NOBJ):
                nc.sync.dma_start(wa1dr[:, n, :, :], wa1dr_d[n, :, :, :])
                nc.sync.dma_start(wtail[:, n, :, :], wtail_d[n, :, :, :])
            wp0 = wtile("wp0", [128, 3, 256], bf16, wp0_d[:, :, :])
            wp0t = wtile("wp0t", [128, 2, 256], f8, wp0t_d[:, :, :])
            wp1 = wtile("wp1", [128, 2, 256], bf16, wp1_d[:, :, :])
            wp2 = wtile("wp2", [128, 2, 8], bf16, wp2_d[:, :, :])
            # zeros accumulator seed: lets object 0's pi init run on DVE via
            # the same fused relu-add op, keeping ACT's queue short (ACT
            # completions gate the a0 h-bank reuse chain)
            zer = s_w.tile([128, 3, TN], bf16, tag="zer")
            nc.gpsimd.memset(zer[:], 0.0)

            pi_store = {}    # t -> pi tile [128,3,TN] bf16
            tt_store = {}    # t -> ttail tile [128,2,TN] f8
            p0T_store = {}   # t -> p0T
            p1T_store = {}   # t -> p1T
            xg_store = {}    # t -> [xg0..xg3]
            hT_store = {}    # (t, pair) -> hT [128,4,TN] f8

            # ---- attention/gating for tile t
            def emit_att_block(t, gate_on_dve=False):
                if t in pre_in:
                    xt, gt = pre_in.pop(t)
                else:
                    xt, gt = emit_in_dma(t)
                att_ps = p_s.tile([128, TN], f32, tag="misc")
                nc.tensor.matmul(att_ps[:], wcast[:], gt[:], start=True,
                                 stop=True)
                attr = s_x.tile([128, TN], bf16, tag="attr")
                nc.scalar.activation(attr[:], att_ps[:], AF.Sigmoid,
                                     bias=bvec[:, 0:1])
                xg = []
                for p in range(4):
                    xgp = s_x.tile([128, TN], bf16, tag=f"xg{p}")
                    if gate_on_dve:
                        nc.vector.tensor_tensor(xgp[:], xt[:, p, :], attr[:],
                                                ALU.mult)
                    else:
                        nc.gpsimd.tensor_tensor(xgp[:], xt[:, p, :], attr[:],
                                                ALU.mult)
                    xg.append(xgp)
                xg_store[t] = xg

            # ---- one a0 chunk: both row-tiled halves of chunk c for pair p.
            # hT slot layout: [evenObj-kc0, oddObj-kc0, evenObj-kc1, oddObj-kc1]
            # so chunk-c relu writes the contiguous slot pair 2c:2c+2, and
            # object `half`'s DR rhs is hT[:, ds(half, 2, 2), :].
            def emit_a0_chunk(t, p, c, on_act):
                xg = xg_store[t]
                if c == 0:
                    hT = s_h.tile([128, 4, TN], f8, tag="hT")
                    hT_store[(t, p)] = hT
                else:
                    hT = hT_store[(t, p)]
                hc = p_h.tile([128, 2, TN], f32, tag="hc")
                nc.tensor.matmul(hc[:, 0, :], wa0[0:51, c, :],
                                 xg[p][0:51, :], start=True, stop=True)
                nc.tensor.matmul(hc[:, 1, :], wa0[64:115, c, :],
                                 xg[p][64:115, :], start=True, stop=True)
                if on_act:
                    nc.scalar.activation(hT[:, 2 * c:2 * c + 2, :], hc[:],
                                         AF.Relu, scale=H_S)
                else:
                    nc.vector.tensor_scalar(hT[:, 2 * c:2 * c + 2, :], hc[:],
                                            0.0, H_S, ALU.max, ALU.mult)

            # ---- deferred p-chain: tile t's tail emitted during tile t+2
            def emit_p0(t, alt_bank=False):
                pi = pi_store.pop(t)
                ttail = tt_store.pop(t)
                p0T = s_p.tile([128, 2, TN], bf16, tag="p0T")
                for c in range(2):
                    mw = ds(128 * c, 128)
                    if alt_bank:
                        p0_ps2 = p_yA.tile([128, 1, TN], f32, tag="yA")
                        p0_ps = p0_ps2[:, 0, :]
                    else:
                        p0_ps = p_s.tile([128, TN], f32, tag="misc")
                    nc.tensor.matmul(p0_ps[:], wp0[:, 0, mw], pi[:, 0, :],
                                     start=True, stop=False)
                    nc.tensor.matmul(p0_ps[:], wp0[:, 1, mw], pi[:, 1, :],
                                     start=False, stop=False)
                    nc.tensor.matmul(p0_ps[:], wp0[:, 2, mw], pi[:, 2, :],
                                     start=False, stop=False)
                    nc.tensor.matmul(p0_ps[:], wp0t[:, :, mw], ttail[:, :, :],
                                     start=False, stop=True, perf_mode=DR)
                    if zero_bp:
                        nc.scalar.activation(p0T[:, c, :], p0_ps[:], AF.Relu,
                                             scale=1.0 / WP0_S)
                    else:
                        nc.scalar.activation(p0T[:, c, :], p0_ps[:], AF.Relu,
                                             scale=1.0 / WP0_S,
                                             bias=bvec[:, 5 + c:6 + c])
                p0T_store[t] = p0T

            def emit_p1(t, alt_bank=False):
                p0T = p0T_store.pop(t)
                p1T = s_p.tile([128, 2, TN], bf16, tag="p1T")
                for c in range(2):
                    mw = ds(128 * c, 128)
                    if alt_bank:
                        p1_ps2 = p_yA.tile([128, 1, TN], f32, tag="yA")
                        p1_ps = p1_ps2[:, 0, :]
                    else:
                        p1_ps = p_s.tile([128, TN], f32, tag="misc")
                    nc.tensor.matmul(p1_ps[:], wp1[:, 0, mw], p0T[:, 0, :],
                                     start=True, stop=False)
                    nc.tensor.matmul(p1_ps[:], wp1[:, 1, mw], p0T[:, 1, :],
                                     start=False, stop=True)
                    if zero_bp:
                        nc.scalar.activation(p1T[:, c, :], p1_ps[:], AF.Relu)
                    else:
                        nc.scalar.activation(p1T[:, c, :], p1_ps[:], AF.Relu,
                                             bias=bvec[:, 7 + c:8 + c])
                p1T_store[t] = p1T

            def emit_p2(t, alt_bank=False):
                p1T = p1T_store.pop(t)
                cs = ds(t * TN, TN)
                if alt_bank:
                    o_ps2 = p_yA.tile([128, 1, TN], f32, tag="yA")
                    o_ps = o_ps2[:, 0, :]
                else:
                    o_ps = p_s.tile([128, TN], f32, tag="misc")
                nc.tensor.matmul(o_ps[0:8, :], wp2[:, 0, :], p1T[:, 0, :],
                                 start=True, stop=False)
                nc.tensor.matmul(o_ps[0:8, :], wp2[:, 1, :], p1T[:, 1, :],
                                 start=False, stop=True)
                ot = s_o.tile([8, TN], f32, tag="ot")
                nc.scalar.activation(ot[:], o_ps[0:8, :], AF.Tanh,
                                     bias=bvec[0:8, 9:10])
                nc.sync.dma_start(out_d[:, cs], ot[:])

            # ---- main stream: tile 0's gating runs on the (idle) DVE so
            # the first objects aren't serialized behind slow gpsimd TTs
            emit_att_block(0, gate_on_dve=True)
            emit_att_block(1)
            # a0 prologue for tile 0: pair 0 fully, pair 1 chunk 0
            emit_a0_chunk(0, 0, 0, True)
            emit_a0_chunk(0, 0, 1, True)
            emit_a0_chunk(0, 1, 0, True)
            for t in range(nt):
                pi = s_pi.tile([128, 3, TN], bf16, tag="pi")
                ttail = s_tt.tile([128, 2, TN], f8, tag="ttail")
                for n in range(NOBJ):
                    p, half = n >> 1, n & 1
                    hT = hT_store[(t, p)]
                    rhs0 = hT[:, half, :]          # kc0
                    rhs1 = hT[:, 2 + half, :]      # kc1
                    yA = p_yA.tile([128, 1, TN], f32, tag="yA")
                    yAB = p_yAB.tile([128, 2, TN], f32, tag="yAB")
                    # cols 0:128 bf16 (2 MMs); 128:256 and 256:384 one
                    # fp8-DR pass each (balanced per-object weight copies)
                    rhs_dr = hT[:, ds(half, 2, 2), :]
                    # yA MMs first for every object: its bank is
                    # single-buffered and its DVE op is the short one, so
                    # write-early + drain-early frees it well before the
                    # next object's yA MMs; the 2-buffered yAB banks absorb
                    # the long 2-slab DVE op instead.
                    mms = [
                        (yA[:, 0, :], wa1[:, 0, :], rhs0, True, False, None),
                        (yA[:, 0, :], wa1[:, 1, :], rhs1, False, True, None),
                        (yAB[:, 0, :], wa1dr[:, n, :, 0:128], rhs_dr,
                         True, True, DR),
                        (yAB[:, 1, :], wa1dr[:, n, :, 128:256], rhs_dr,
                         True, True, DR),
                    ]
                    for out_ap, w_ap, r_ap, st, sp, pm in mms:
                        nc.tensor.matmul(out_ap, w_ap, r_ap, start=st,
                                         stop=sp, perf_mode=pm)
                    # pi accumulation (fused relu+add on DVE)
                    if zero_ba1:
                        in1B = zer[:, 1:3, :] if n == 0 else pi[:, 1:3, :]
                        in1A = zer[:, 0:1, :] if n == 0 else pi[:, 0:1, :]
                        nc.vector._custom_dve(ra_op, out=pi[:, 0:1, :],
                                              in0=yA[:], in1=in1A,
                                              s0=Y_DESCALE)
                        nc.vector._custom_dve(ra_op, out=pi[:, 1:3, :],
                                              in0=yAB[:], in1=in1B,
                                              s0=Y_DESCALE)
                    else:
                        tt = s_x.tile([128, 3, TN], bf16, tag="ttmp")
                        for bk in range(3):
                            src = yA[:, 0, :] if bk == 0 else yAB[:, bk - 1, :]
                            nc.scalar.activation(tt[:, bk, :], src, AF.Relu,
                                                 scale=Y_DESCALE,
                                                 bias=bvec[:, 1 + bk:2 + bk])
                        if n == 0:
                            nc.vector.tensor_copy(pi[:], tt[:])
                        else:
                            nc.vector.tensor_tensor(pi[:], pi[:], tt[:],
                                                    ALU.add)
                    # interleaved deferred work
                    if n < 5:
                        q = n + 3
                        emit_a0_chunk(t, (q >> 1) % 4, q & 1, on_act=True)
                        if n == 3 and t + 2 < nt:
                            # att for t+2 emitted mid-tile so gating isn't
                            # just-in-time for t+2's a0 matmuls
                            emit_att_block(t + 2)
                    elif n == 5:
                        if t + 1 < nt:
                            emit_a0_chunk(t + 1, 0, 0, True)
                    elif n == 6:
                        if t + 1 < nt:
                            emit_a0_chunk(t + 1, 0, 1, True)
                        if t >= 1:
                            emit_p2(t - 1)
                    elif n == 7:
                        if t + 1 < nt:
                            emit_a0_chunk(t + 1, 1, 0, True)
                    # batched packed tails (emitted after the a0-chunk so the
                    # low-urgency tail relu sits behind the MM-gating a0 relu
                    # in ACT's FIFO)
                    if n == 2 or n == 7:
                        tb = n >> 2
                        Tm = p_s.tile([128, TN], f32, tag="misc")
                        for kc in range(2):
                            for k in range(4):
                                nobj = 4 * tb + k
                                hTk = hT_store[(t, nobj >> 1)]
                                rhs = hTk[:, 2 * kc + (nobj & 1), :]
                                nc.tensor.matmul(Tm[32 * k:32 * k + 32, :],
                                                 wtail[:, nobj, kc, :], rhs,
                                                 start=kc == 0, stop=kc == 1,
                                                 tile_position=(0, 32 * k),
                                                 skip_group_check=True)
                        if zero_ba1:
                            nc.scalar.activation(ttail[:, tb, :], Tm[:],
                                                 AF.Relu, scale=TT_S / 64.0)
                        else:
                            nc.scalar.activation(ttail[:, tb, :], Tm[:],
                                                 AF.Relu, scale=TT_S / 64.0,
                                                 bias=bvec[:, 4:5])
                    if n == 1 and t >= 1:
                        emit_p0(t - 1)
                    elif n == 4 and t >= 1:
                        emit_p1(t - 1)

                    if n == 7:
                        for pp in range(4):
                            hT_store.pop((t, pp), None)
                pi_store[t] = pi
                tt_store[t] = ttail

            # ---- epilogue: p-chain for the last tile
            emit_p0(nt - 1, alt_bank=True)
            emit_p1(nt - 1, alt_bank=True)
            emit_p2(nt - 1, alt_bank=True)

    nc.compile()
    return nc


# ---------------------------------------------------------------- entry point

_DEV_WEIGHT_KEYS = ("wcast", "bvec", "wa0", "wa1", "wp0", "wp0t", "wp1",
                    "wp2")


def _prep_in_maps(o, g, weights):
    o = np.asarray(o, np.float32)
    g = np.asarray(g, np.float32)
    in_maps = []
    for c in range(NCORES):
        sl = slice(c * BSH, (c + 1) * BSH)
        m = {k: weights[k] for k in _DEV_WEIGHT_KEYS}
        m["wa1dr"] = weights["wa1dr"]
        m["wtail"] = weights["wtail"]
        m.update(_pack_shard(o[sl], g[sl]))
        in_maps.append(m)
    return in_maps


def run(o, g, W_cast, b_cast, W_a0, b_a0, W_a1, b_a1,
        W_p0, b_p0, W_p1, b_p1, W_p2, b_p2, trace=False):
    from concourse.bass_utils import run_bass_kernel_spmd
    args = [np.asarray(a, np.float32) for a in
            (W_cast, b_cast, W_a0, b_a0, W_a1, b_a1, W_p0, b_p0, W_p1, b_p1,
             W_p2, b_p2)]
    weights = _pack_weights(*args)
    zero_ba1 = not np.any(args[5])
    zero_bp = not (np.any(args[7]) or np.any(args[9]))
    nc = build_nc(BSH, zero_ba1=zero_ba1, zero_bp=zero_bp)
    in_maps = _prep_in_maps(o, g, weights)
    res = run_bass_kernel_spmd(nc, in_maps, core_ids=list(range(NCORES)),
                               trace=trace)
    outs = [np.asarray(res.results[c]["out"], np.float32).T
            for c in range(NCORES)]
    return np.concatenate(outs, axis=0), res


def kernel(**inputs):
    out, _ = run(**inputs)
    return out

